# revision 5
# baseline (speedup 1.0000x reference)
"""Int8RouterLinear TRN2 kernel: out[16384, 64] = x[16384, 4096] @ (W_int8 * scale)^T.

Strategy (data-parallel over 8 NeuronCores, 2048 tokens each):
  - The host pre-transposes and fp16-casts each core's x shard into
    [128p, 32k, 2048t] tile layout (h = 128k + p on partitions), so the
    device does a pure streaming matmul — no on-chip transposes.
  - x streams HBM->SBUF over both HWDGE rings with tapered transfer
    sizes ([8,4,2,1,1] h-tiles per ring, 4KB partition lines): big
    transfers early amortize per-DMA overhead, small ones at the end
    shorten the dependency tail. 16MB/core = the memory roofline.
  - Every transfer lands in its own SBUF tile (16MB resident), so DMA
    never stalls on buffer recycling.
  - The dequantized router weight is stationary: fp16 keeps the int8
    weight values exact, so only the scale product and x round to 11
    bits -> ~3e-4 rel err (measured; gate is 2e-2). Each ring carries
    half the weight first so the PE can start early.
  - matmul accumulates out^T[64e, 512t] chunks in fp32 PSUM over the 32
    h-tiles (4 PSUM banks, one per 512-token chunk of the shard).
  - Tail: per-chunk PSUM->SBUF copies alternate DVE/ACT engines, each
    chased by its own 128KB store on an alternating ring.
"""
import numpy as np

import concourse.mybir as mybir
from concourse import bacc
from concourse.tile import TileContext
from concourse.bass_utils import run_bass_kernel_spmd

TOKENS = 16384
HIDDEN = 4096
EXPERTS = 64
NCORES = 8
TSHARD = TOKENS // NCORES          # 2048 tokens per core
HT = HIDDEN // 128                 # 32 h-tiles of 128
CHUNK = 512                        # tokens per PSUM accumulation bank
NCHUNK = TSHARD // CHUNK           # 4
TAPER = [8, 4, 2, 1, 1]            # h-tiles per transfer, per ring

F32 = mybir.dt.float32
F16 = mybir.dt.float16

_cache = {}


def _blocks():
    """(k0, nh, ring) DMA blocks: taper sizes, k-order, alternating rings."""
    out = []
    k0 = 0
    for sz in TAPER:
        for ring in (0, 1):
            out.append((k0, sz, ring))
            k0 += sz
    assert k0 == HT
    return out


def _build():
    if "nc" in _cache:
        return _cache["nc"]

    nc = bacc.Bacc("TRN2", target_bir_lowering=False, debug=False,
                   num_devices=NCORES)
    x_d = nc.dram_tensor("x", [128, HT, TSHARD], F16, kind="ExternalInput")
    w_d = nc.dram_tensor("w", [128, HT * EXPERTS], F16, kind="ExternalInput")
    o_d = nc.dram_tensor("out", [EXPERTS, TSHARD], F32, kind="ExternalOutput")
    x = x_d.ap()

    with TileContext(nc) as tc:
        with tc.tile_pool(name="consts", bufs=1) as cpool, \
             tc.tile_pool(name="xp", bufs=1) as xpool, \
             tc.tile_pool(name="pso", bufs=1, space="PSUM") as ppool, \
             tc.tile_pool(name="ost", bufs=1) as opool:
            HALF = HT // 2 * EXPERTS
            w_sb = [cpool.tile([128, HALF], F16, name=f"w{h}", tag=f"w{h}")
                    for h in range(2)]
            nc.sync.dma_start(out=w_sb[0], in_=w_d.ap()[:, :HALF])
            nc.scalar.dma_start(out=w_sb[1], in_=w_d.ap()[:, HALF:])
            w_v = [t.rearrange("p (k e) -> p k e", e=EXPERTS) for t in w_sb]

            po = [ppool.tile([EXPERTS, CHUNK], F32, name=f"po{c}",
                             tag=f"po{c}") for c in range(NCHUNK)]

            rings = [nc.sync, nc.scalar]
            for bi, (k0, nh, ring) in enumerate(_blocks()):
                xg = xpool.tile([128, nh * TSHARD], F16, name=f"x{bi}",
                                tag=f"x{bi}")
                rings[ring].dma_start(out=xg, in_=x[:, k0:k0 + nh, :])
                xv = xg.rearrange("p (k t) -> p k t", k=nh)
                for j in range(nh):
                    k = k0 + j
                    wt = w_v[k // (HT // 2)][:, k % (HT // 2), :]
                    for c in range(NCHUNK):
                        nc.tensor.matmul(
                            po[c], wt, xv[:, j, c * CHUNK:(c + 1) * CHUNK],
                            start=(k == 0), stop=(k == HT - 1))

            # tail pipeline: per-chunk PSUM->SBUF copy (alternating DVE/ACT)
            # chased by its own 128KB store on an alternating ring.
            ot = opool.tile([EXPERTS, TSHARD], F32)
            for c in range(NCHUNK):
                sl = slice(c * CHUNK, (c + 1) * CHUNK)
                if c % 2 == 0:
                    nc.vector.tensor_copy(ot[:, sl], po[c])
                else:
                    nc.scalar.copy(ot[:, sl], po[c])
                rings[(c + 1) % 2].dma_start(out=o_d.ap()[:, sl], in_=ot[:, sl])

    nc.compile()
    _cache["nc"] = nc
    return nc


def _prep_w(weights_int8, scales):
    """[64, 4096] int8-valued weights -> [128, HT*EXPERTS] fp16 with
    w_arr[p, k*64 + e] = W[e, 128k + p]."""
    w = weights_int8.astype(np.float32) * scales.astype(np.float32)[:, None]
    wt = w.T.astype(np.float16)                      # [HIDDEN, EXPERTS]
    arr = wt.reshape(HT, 128, EXPERTS).transpose(1, 0, 2)
    return np.ascontiguousarray(arr).reshape(128, HT * EXPERTS)


def _prep_x(x):
    """Transpose + fp16-cast x into per-core [128, HT, TSHARD] arrays with
    x_c[p, k, t] = x[c*TSHARD + t, 128k + p]."""
    x16 = x.astype(np.float16)
    xt = np.empty((HIDDEN, TOKENS), dtype=np.float16)
    blk = 512
    for i in range(0, TOKENS, blk):                  # blocked: cache-friendly
        xt[:, i:i + blk] = x16[i:i + blk].T
    shards = []
    for c in range(NCORES):
        sh = xt[:, c * TSHARD:(c + 1) * TSHARD]
        sh = sh.reshape(HT, 128, TSHARD).transpose(1, 0, 2)
        shards.append(np.ascontiguousarray(sh))
    return shards


def kernel(x, weights_int8, scales):
    nc = _build()
    warr = _prep_w(np.asarray(weights_int8), np.asarray(scales))
    xshards = _prep_x(np.ascontiguousarray(x, dtype=np.float32))
    in_maps = [{"x": xshards[c], "w": warr} for c in range(NCORES)]
    res = run_bass_kernel_spmd(nc, in_maps, core_ids=list(range(NCORES)))
    out = np.concatenate(
        [res.results[c]["out"].T for c in range(NCORES)], axis=0)
    return np.ascontiguousarray(out, dtype=np.float32)


# revision 7
# speedup vs baseline: 1.0287x; 1.0287x over previous
"""Int8RouterLinear TRN2 kernel: out[16384, 64] = x[16384, 4096] @ (W_int8 * scale)^T.

Strategy (data-parallel over 8 NeuronCores, 2048 tokens each):
  - The host pre-transposes and fp16-casts each core's x shard into
    [128p, 32k, 2048t] tile layout (h = 128k + p on partitions), so the
    device does a pure streaming matmul — no on-chip transposes.
  - x streams HBM->SBUF over both HWDGE rings with tapered transfer
    sizes ([8,4,2,1,1] h-tiles per ring, 4KB partition lines): big
    transfers early amortize per-DMA overhead, small ones at the end
    shorten the dependency tail. 16MB/core = the memory roofline.
  - Every transfer lands in its own SBUF tile (16MB resident), so DMA
    never stalls on buffer recycling.
  - The dequantized router weight is stationary: fp16 keeps the int8
    weight values exact, so only the scale product and x round to 11
    bits -> ~3e-4 rel err (measured; gate is 2e-2). Each ring carries
    half the weight first so the PE can start early.
  - matmul accumulates out^T[64e, 512t] chunks in fp32 PSUM over the 32
    h-tiles (4 PSUM banks, one per 512-token chunk of the shard).
  - Tail: per-chunk PSUM->SBUF copies alternate DVE/ACT engines, each
    chased by its own 128KB store on an alternating ring.
"""
import numpy as np

import concourse.mybir as mybir
from concourse import bacc
from concourse.tile import TileContext
from concourse.bass_utils import run_bass_kernel_spmd

TOKENS = 16384
HIDDEN = 4096
EXPERTS = 64
NCORES = 8
TSHARD = TOKENS // NCORES          # 2048 tokens per core
HT = HIDDEN // 128                 # 32 h-tiles of 128
CHUNK = 512                        # tokens per PSUM accumulation bank
NCHUNK = TSHARD // CHUNK           # 4
TAPER = [4, 4, 3, 2, 1, 1, 1]      # h-tiles per transfer, per ring (decreasing)

F32 = mybir.dt.float32
F16 = mybir.dt.float16

_cache = {}


def _blocks():
    """(k0, nh, ring) DMA blocks: decreasing taper, k-order, alternating
    rings so blocks complete in k order (the PE consumes strictly in
    program order) and the final h-tiles land early and staggered."""
    out = []
    k0 = 0
    for sz in TAPER:
        for ring in (0, 1):
            out.append((k0, sz, ring))
            k0 += sz
    assert k0 == HT
    return out


def _build():
    if "nc" in _cache:
        return _cache["nc"]

    nc = bacc.Bacc("TRN2", target_bir_lowering=False, debug=False,
                   num_devices=NCORES)
    x_d = nc.dram_tensor("x", [128, HT, TSHARD], F16, kind="ExternalInput")
    w_d = nc.dram_tensor("w", [128, HT * EXPERTS], F16, kind="ExternalInput")
    o_d = nc.dram_tensor("out", [EXPERTS, TSHARD], F32, kind="ExternalOutput")
    x = x_d.ap()

    with TileContext(nc) as tc:
        with tc.tile_pool(name="consts", bufs=1) as cpool, \
             tc.tile_pool(name="xp", bufs=1) as xpool, \
             tc.tile_pool(name="pso", bufs=1, space="PSUM") as ppool, \
             tc.tile_pool(name="ost", bufs=1) as opool:
            HALF = HT // 2 * EXPERTS
            w_sb = [cpool.tile([128, HALF], F16, name=f"w{h}", tag=f"w{h}")
                    for h in range(2)]
            nc.sync.dma_start(out=w_sb[0], in_=w_d.ap()[:, :HALF])
            nc.scalar.dma_start(out=w_sb[1], in_=w_d.ap()[:, HALF:])
            w_v = [t.rearrange("p (k e) -> p k e", e=EXPERTS) for t in w_sb]

            po = [ppool.tile([EXPERTS, CHUNK], F32, name=f"po{c}",
                             tag=f"po{c}") for c in range(NCHUNK)]

            rings = [nc.sync, nc.scalar]
            for bi, (k0, nh, ring) in enumerate(_blocks()):
                xg = xpool.tile([128, nh * TSHARD], F16, name=f"x{bi}",
                                tag=f"x{bi}")
                rings[ring].dma_start(out=xg, in_=x[:, k0:k0 + nh, :])
                xv = xg.rearrange("p (k t) -> p k t", k=nh)
                for j in range(nh):
                    k = k0 + j
                    wt = w_v[k // (HT // 2)][:, k % (HT // 2), :]
                    for c in range(NCHUNK):
                        nc.tensor.matmul(
                            po[c], wt, xv[:, j, c * CHUNK:(c + 1) * CHUNK],
                            start=(k == 0), stop=(k == HT - 1))

            # tail pipeline: per-chunk PSUM->SBUF copy (alternating DVE/ACT)
            # chased by its own 128KB store on an alternating ring.
            ot = opool.tile([EXPERTS, TSHARD], F32)
            for c in range(NCHUNK):
                sl = slice(c * CHUNK, (c + 1) * CHUNK)
                if c % 2 == 0:
                    nc.vector.tensor_copy(ot[:, sl], po[c])
                else:
                    nc.scalar.copy(ot[:, sl], po[c])
                rings[(c + 1) % 2].dma_start(out=o_d.ap()[:, sl], in_=ot[:, sl])

    nc.compile()
    _cache["nc"] = nc
    return nc


def _prep_w(weights_int8, scales):
    """[64, 4096] int8-valued weights -> [128, HT*EXPERTS] fp16 with
    w_arr[p, k*64 + e] = W[e, 128k + p]."""
    w = weights_int8.astype(np.float32) * scales.astype(np.float32)[:, None]
    wt = w.T.astype(np.float16)                      # [HIDDEN, EXPERTS]
    arr = wt.reshape(HT, 128, EXPERTS).transpose(1, 0, 2)
    return np.ascontiguousarray(arr).reshape(128, HT * EXPERTS)


def _prep_x(x):
    """Transpose + fp16-cast x into per-core [128, HT, TSHARD] arrays with
    x_c[p, k, t] = x[c*TSHARD + t, 128k + p]."""
    x16 = x.astype(np.float16)
    xt = np.empty((HIDDEN, TOKENS), dtype=np.float16)
    blk = 512
    for i in range(0, TOKENS, blk):                  # blocked: cache-friendly
        xt[:, i:i + blk] = x16[i:i + blk].T
    shards = []
    for c in range(NCORES):
        sh = xt[:, c * TSHARD:(c + 1) * TSHARD]
        sh = sh.reshape(HT, 128, TSHARD).transpose(1, 0, 2)
        shards.append(np.ascontiguousarray(sh))
    return shards


def kernel(x, weights_int8, scales):
    nc = _build()
    warr = _prep_w(np.asarray(weights_int8), np.asarray(scales))
    xshards = _prep_x(np.ascontiguousarray(x, dtype=np.float32))
    in_maps = [{"x": xshards[c], "w": warr} for c in range(NCORES)]
    res = run_bass_kernel_spmd(nc, in_maps, core_ids=list(range(NCORES)))
    out = np.concatenate(
        [res.results[c]["out"].T for c in range(NCORES)], axis=0)
    return np.ascontiguousarray(out, dtype=np.float32)
